# revision 15
# baseline (speedup 1.0000x reference)
"""Multi-head causal attention (B=2, S=2048, D=1024, H=16) on 8 trn2 NeuronCores.

Sharding: data-parallel over batch (2) x tensor-parallel over heads (4 groups of
4 heads).  Core c = 4*b + g handles batch b, heads [4g, 4g+4).  Each core
computes a partial output  ctx_g @ Wo_g.T  [2048, 1024]; the host sums the 4
partials per batch.

Within-core dataflow:
  qT,kT = W @ X.T        float32r matmuls (fp32 operands, fp22 datapath, full
                         PE rate at moving-dim >= 256); [dk, s] pair-packed
  v     = X @ Wv.T       natural [s, dv], stored bf16
  S'    = -8 * (qT.T @ kT + mask)   per 128-row query tile, causally-trimmed
                         1024-wide PSUM chunks; two heads row-packed via
                         tile_position.  The -8 scale + mask add + running
                         row min (= -8*max) come from one chained
                         tensor_tensor_reduce per chunk (DVE).
  P     = exp(-S' - 8m)  one ScalarE activation per chunk (scale=-1,
                         bias = chained min), accum_out gives row sums Z;
                         P written bf16.  Global row max => no rescale pass.
  P    *= 1/Z            per chunk; DVE (4x bf16) for late tiles, GPSIMD for
                         early ones (Pool engine otherwise idle)
  P_T   = PE transpose   bf16 128x128 blocks; both heads packed into one
                         [128,1024] PSUM tile per key tile, copied to SBUF
                         in one wide copy (engine chosen by a build-time
                         DVE/ACT cost balancer)
  ctxT  = v.T @ P_T      bf16, two heads col-packed, accumulated over k tiles
  out  += ctxT.T @ WoT   float32r, per s-tile, stored straight from PSUM
"""

import numpy as np

B, S, D, H = 2, 2048, 1024, 16
DK = D // H          # 64
JC = 256             # per-core projection width (4 heads * 64)
NQT = S // 128       # 16 query tiles
NU4 = S // 512       # 4 query supertiles
_SCALE = float(DK) ** 0.5  # 8.0  (reference multiplies scores by sqrt(dk))
_MASKVAL = -1.0e30
_BIG = 3.0e38
CW = 512              # score chunk width (PSUM bank-local)
PSS_BUFS = 6
USE_TTR = False

_cached = {}
_BUILD_STAGES = "full"  # debug: dma | proj | full


def _build_nc(reps=1):
    stages = _BUILD_STAGES
    from contextlib import ExitStack

    import concourse.mybir as mybir
    import concourse.tile as tile
    from concourse import bacc

    F32 = mybir.dt.float32
    F32R = mybir.dt.float32r
    BF16 = mybir.dt.bfloat16
    EXP = mybir.ActivationFunctionType.Exp
    AX = mybir.AxisListType.X
    MIN = mybir.AluOpType.min
    MAX = mybir.AluOpType.max
    ADD = mybir.AluOpType.add

    nc = bacc.Bacc("TRN2", target_bir_lowering=False)

    xtq_d = nc.dram_tensor("xtq", [D, S], F32R, kind="ExternalInput")
    xtk_d = nc.dram_tensor("xtk", [D, S], F32R, kind="ExternalInput")
    xtv_d = nc.dram_tensor("xtv", [D, S], F32R, kind="ExternalInput")
    wqt_d = nc.dram_tensor("wqt", [D, JC], F32R, kind="ExternalInput")
    wkt_d = nc.dram_tensor("wkt", [D, JC], F32R, kind="ExternalInput")
    wvt_d = nc.dram_tensor("wvt", [D, JC], F32R, kind="ExternalInput")
    wot_d = nc.dram_tensor("wot", [JC, D], F32R, kind="ExternalInput")
    cmask_d = nc.dram_tensor("cmask", [128, 128], F32, kind="ExternalInput")
    ident_d = nc.dram_tensor("ident", [128, 128], BF16, kind="ExternalInput")
    out_d = nc.dram_tensor("out", [S, D], F32, kind="ExternalOutput")

    # build-time engine busy estimate (ns) for greedy op routing.
    # DVE: (120+FD/accel)/0.96 from PSUM; ACT: (172+FD)/1.2 (no 16-bit
    # packing observed on ScalarE); Pool: 95 + 1.67*FD/1.2 SBUF-only.
    bal = {"v": 0.0, "a": 0.0, "p": 0.0}

    def route_copy(dst, src, fd, accel_v, vec, act):
        cv = (120 + fd / accel_v) / 0.96
        ca = (172 + fd) / 1.2
        if bal["v"] + cv <= bal["a"] + ca:
            bal["v"] += cv
            vec(dst, src)
        else:
            bal["a"] += ca
            act(dst, src)

    with tile.TileContext(nc) as tc, ExitStack() as top:
        res = top.enter_context(tc.tile_pool(name="res", bufs=1))
        stats = top.enter_context(tc.tile_pool(name="stats", bufs=1))

        # ---- resident tiles -------------------------------------------------
        # weight layouts: w?_sb[p, dt, j] = W?T[128*dt + p, j]
        wq_sb = res.tile([128, 8, JC], F32R, tag="wq")
        wk_sb = res.tile([128, 8, JC], F32R, tag="wk")
        wv_sb = res.tile([128, 8, JC], F32R, tag="wv")
        nc.sync.dma_start(wq_sb, wqt_d[:, :].rearrange("(t p) j -> p t j", p=128))
        nc.sync.dma_start(wk_sb, wkt_d[:, :].rearrange("(t p) j -> p t j", p=128))
        nc.sync.dma_start(wv_sb, wvt_d[:, :].rearrange("(t p) j -> p t j", p=128))
        wo_sb = []
        for p2 in range(2):
            t = res.tile([128, D], F32R, tag=f"wo{p2}", name=f"wo{p2}")
            nc.sync.dma_start(t, wot_d[128 * p2 : 128 * (p2 + 1), :])
            wo_sb.append(t)
        cmask = res.tile([128, 128], F32, tag="cmask")
        ident = res.tile([128, 128], BF16, tag="ident")
        nc.sync.dma_start(cmask, cmask_d[:, :])
        nc.sync.dma_start(ident, ident_d[:, :])
        zeros = res.tile([128, 1024], F32, tag="zeros")
        nc.scalar.memzero(zeros[:, :])

        # projected tensors (resident through attention), segmented 512-wide so
        # Tile's per-tile dependency tracking lets attention start on early
        # segments while later projection chunks are still in flight
        qseg = [[res.tile([128, 512], F32R, tag=f"qts{i}{c}", name=f"qts{i}{c}")
                 for c in range(4)] for i in range(2)]
        kseg = [[res.tile([128, 512], F32R, tag=f"kts{i}{c}", name=f"kts{i}{c}")
                 for c in range(4)] for i in range(2)]
        vu = [res.tile([128, JC], BF16, tag=f"vu{i}", name=f"vu{i}") for i in range(NQT)]
        ctxseg = [[res.tile([128, 512], F32R, tag=f"ctx{i}{c}", name=f"ctx{i}{c}")
                   for c in range(4)] for i in range(2)]

        def _one_pass(_rep):
            # ---- stage B: projections --------------------------------------
            with ExitStack() as stage_b:
                xpool = stage_b.enter_context(tc.tile_pool(name=f"xt{_rep}", bufs=1))
                pjp = stage_b.enter_context(
                    tc.tile_pool(name=f"pj{_rep}", bufs=1, space="PSUM")
                )
                CH = 512
                work = [(xtq_d, "q", 0), (xtk_d, "k", 0), (xtq_d, "q", 1),
                        (xtk_d, "k", 1), (xtq_d, "q", 2), (xtk_d, "k", 2),
                        (xtq_d, "q", 3), (xtk_d, "k", 3)]
                work += [(xtv_d, "v", c) for c in range(4)]
                for n_, (xd, kind, ch) in enumerate(work):
                    sl = slice(ch * CH, (ch + 1) * CH)
                    xc = xpool.tile([128, 8, CH], F32R, tag="xc", bufs=3, name="xc")
                    dmae = nc.sync if n_ % 2 == 0 else nc.scalar
                    dmae.dma_start(
                        xc, xd[:, sl].rearrange("(t p) s -> p t s", p=128)
                    )
                    if stages == "dma":
                        dd = stats.tile([128, 1], F32, tag="dd", bufs=8, name="dd")
                        nc.vector.reduce_max(
                            out=dd, in_=xc[:, 0, :].bitcast(F32), axis=AX
                        )
                        nc.sync.dma_start(out_d[0:128, n_ : n_ + 1], dd)
                        continue
                    if kind in ("q", "k"):
                        wsb = wq_sb if kind == "q" else wk_sb
                        dst = qseg if kind == "q" else kseg
                        for jt in range(2):
                            ps = pjp.tile([128, CH], F32, tag="pj", bufs=3,
                                          name="psqk")
                            for dt in range(8):
                                nc.tensor.matmul(
                                    ps,
                                    wsb[:, dt, 128 * jt : 128 * (jt + 1)],
                                    xc[:, dt, :],
                                    start=(dt == 0),
                                    stop=(dt == 7),
                                )
                            route_copy(dst[jt][ch], ps, 512, 1,
                                       nc.vector.tensor_copy, nc.scalar.copy)
                    else:
                        for st in range(4):
                            ps = pjp.tile([128, JC], F32, tag="pjv", bufs=2,
                                          name="psv")
                            for dt in range(8):
                                nc.tensor.matmul(
                                    ps,
                                    xc[:, dt, st * 128 : (st + 1) * 128],
                                    wv_sb[:, dt, :],
                                    start=(dt == 0),
                                    stop=(dt == 7),
                                )
                            route_copy(vu[4 * ch + st], ps, 256, 2,
                                       nc.vector.tensor_copy, nc.scalar.copy)

            if stages == "dma":
                return
            if stages == "proj":
                nc.sync.dma_start(out_d[0:128, :], qseg[0][0][:, 0:1024].bitcast(F32))
                nc.sync.dma_start(out_d[128:256, :], kseg[1][0][:, 0:1024].bitcast(F32))
                return

            # ---- stage C/D: attention + output projection ------------------
            with ExitStack() as stage_c:
                ppool = stage_c.enter_context(tc.tile_pool(name=f"pp{_rep}", bufs=1))
                ptp = stage_c.enter_context(tc.tile_pool(name=f"ptp{_rep}", bufs=1))
                obp = stage_c.enter_context(tc.tile_pool(name=f"obp{_rep}", bufs=1))
                pss_p = stage_c.enter_context(
                    tc.tile_pool(name=f"pss{_rep}", bufs=1, space="PSUM"))
                pst_p = stage_c.enter_context(
                    tc.tile_pool(name=f"pst{_rep}", bufs=1, space="PSUM"))
                psc_p = stage_c.enter_context(
                    tc.tile_pool(name=f"psc{_rep}", bufs=1, space="PSUM"))

                for u in range(NU4):
                    for p in range(2):
                        ptiles = {}
                        for sq in range(4):
                            qi = 4 * u + sq
                            W = 128 * (qi + 1)          # causal row width
                            nch = (W + CW - 1) // CW    # CW-wide chunks
                            for h in range(2):
                                hsl = slice(64 * h, 64 * (h + 1))
                                pt = ppool.tile([128, S], BF16, tag="P", bufs=16,
                                                name=f"P{p}{sq}{h}")
                                ptiles[(h, sq)] = pt
                                stt = stats.tile([128, 12], F32, tag="st", bufs=16,
                                                 name="stt")
                                chunks = []
                                k = 0  # running-min chain slot
                                for c in range(nch):
                                    off = CW * c
                                    wc = min(W - off, CW)
                                    ps = pss_p.tile([128, CW], F32, tag="pss",
                                                    bufs=PSS_BUFS, name="pss")
                                    chunks.append((ps, off, wc))
                                    for j in range((wc + 511) // 512):
                                        soff = off + 512 * j
                                        wj = min(wc - 512 * j, 512)
                                        nc.tensor.matmul(
                                            ps[:, 512 * j : 512 * j + wj],
                                            qseg[p][u][hsl, 128 * sq : 128 * sq + 128],
                                            kseg[p][soff // 512][hsl, 0:wj],
                                            start=True,
                                            stop=True,
                                            tile_position=(64 * h, 0),
                                        )
                                    # fold mask + *(-8) + running row-min
                                    # (= -8*rowmax) into chained TTRs
                                    last = c == nch - 1
                                    ndw = wc - 128 if last else wc
                                    if USE_TTR:
                                        if ndw > 0:
                                            nc.vector.tensor_tensor_reduce(
                                                out=ps[:, 0:ndw], in0=ps[:, 0:ndw],
                                                in1=zeros[:, 0:ndw], scale=-_SCALE,
                                                scalar=(_BIG if k == 0
                                                        else stt[:, k - 1 : k]),
                                                op0=ADD, op1=MIN,
                                                accum_out=stt[:, k : k + 1])
                                            bal["v"] += (120 + ndw) / 0.96
                                            k += 1
                                        if last:
                                            nc.vector.tensor_tensor_reduce(
                                                out=ps[:, ndw:wc], in0=ps[:, ndw:wc],
                                                in1=cmask, scale=-_SCALE,
                                                scalar=(_BIG if k == 0
                                                        else stt[:, k - 1 : k]),
                                                op0=ADD, op1=MIN,
                                                accum_out=stt[:, k : k + 1])
                                            bal["v"] += (120 + 128) / 0.96
                                            k += 1
                                    else:
                                        if last:
                                            nc.vector.tensor_add(
                                                ps[:, ndw:wc], ps[:, ndw:wc],
                                                cmask)
                                            bal["v"] += (120 + 128) / 0.96
                                        nc.vector.reduce_max(
                                            out=stt[:, k : k + 1],
                                            in_=ps[:, 0:wc], axis=AX)
                                        bal["v"] += (120 + wc) / 0.96
                                        if k > 0:
                                            nc.vector.tensor_tensor(
                                                out=stt[:, k : k + 1],
                                                in0=stt[:, k - 1 : k],
                                                in1=stt[:, k : k + 1],
                                                op=MAX)
                                        k += 1
                                if USE_TTR:
                                    mfin = stt[:, k - 1 : k]
                                    esc = -1.0
                                else:
                                    nc.vector.tensor_scalar_mul(
                                        stt[:, k : k + 1], stt[:, k - 1 : k],
                                        -_SCALE)
                                    mfin = stt[:, k : k + 1]
                                    esc = _SCALE
                                for c, (ps, off, wc) in enumerate(chunks):
                                    nc.scalar.activation(
                                        out=pt[:, off : off + wc],
                                        in_=ps[:, 0:wc],
                                        func=EXP,
                                        bias=mfin,
                                        scale=esc,
                                        accum_out=stt[:, 5 + c : 6 + c],
                                    )
                                    bal["a"] += (172 + wc + 187) / 1.2
                                if nch > 1:
                                    nc.vector.reduce_sum(
                                        out=stt[:, 9:10], in_=stt[:, 5 : 5 + nch],
                                        axis=AX)
                                    zsum = stt[:, 9:10]
                                else:
                                    zsum = stt[:, 5:6]
                                nc.vector.reciprocal(stt[:, 10:11], zsum)
                                rz = stt[:, 10:11]
                                bal["v"] += 130.0
                                for c, (ps, off, wc) in enumerate(chunks):
                                    cv = (58 + wc / 4) / 0.96
                                    cp = 95 + 1.67 * wc / 1.2
                                    if bal["p"] + cp <= bal["v"] + cv:
                                        bal["p"] += cp
                                        nc.gpsimd.tensor_scalar_mul(
                                            pt[:, off : off + wc],
                                            pt[:, off : off + wc], rz)
                                    else:
                                        bal["v"] += cv
                                        nc.vector.tensor_scalar_mul(
                                            pt[:, off : off + wc],
                                            pt[:, off : off + wc], rz)

                        # transposes + PV for this (pair, supertile)
                        psc = psc_p.tile([128, 512], F32, tag="psc", bufs=1,
                                         name=f"psc{p}{u}")
                        for t in range(4 * u + 4):
                            vstart = max(0, t - 4 * u)
                            pstile = pst_p.tile([128, 1024], BF16, tag="pst",
                                                bufs=1, name="pst")
                            for h in range(2):
                                for sq in range(vstart, 4):
                                    nc.tensor.transpose(
                                        pstile[:, 512 * h + 128 * sq
                                               : 512 * h + 128 * (sq + 1)],
                                        ptiles[(h, sq)][:, 128 * t : 128 * (t + 1)],
                                        ident,
                                    )
                            ptsb = ptp.tile([128, 1024], BF16, tag="pt", bufs=3,
                                            name="ptsb")
                            if vstart == 0:
                                route_copy(ptsb, pstile, 1024, 2,
                                           nc.vector.tensor_copy, nc.scalar.copy)
                            else:
                                for h in range(2):
                                    csl = slice(512 * h + 128 * vstart,
                                                512 * h + 512)
                                    route_copy(ptsb[:, csl], pstile[:, csl],
                                               512 - 128 * vstart, 2,
                                               nc.vector.tensor_copy,
                                               nc.scalar.copy)
                            for h in range(2):
                                csl = slice(128 * vstart, 512)
                                nc.tensor.matmul(
                                    psc[64 * h : 64 * (h + 1), csl],
                                    vu[t][:, 64 * (2 * p + h) : 64 * (2 * p + h + 1)],
                                    ptsb[:, 512 * h + 128 * vstart : 512 * h + 512],
                                    start=(t == 0),
                                    stop=(t == 4 * u + 3),
                                    tile_position=(0, 64 * h),
                                    skip_group_check=True,
                                )
                        route_copy(ctxseg[p][u], psc, 512, 1,
                                   nc.vector.tensor_copy, nc.scalar.copy)

                    # output projection for the four finished s-tiles
                    for st_ in range(4 * u, 4 * u + 4):
                        ssl = slice(128 * st_, 128 * (st_ + 1))
                        csl_ = slice(128 * (st_ % 4), 128 * (st_ % 4) + 128)
                        for oc in range(2):
                            osl = slice(512 * oc, 512 * (oc + 1))
                            pso = pss_p.tile([128, CW], F32, tag="pss",
                                             bufs=PSS_BUFS, name="pso")
                            nc.tensor.matmul(pso[:, 0:512], ctxseg[0][u][:, csl_],
                                             wo_sb[0][:, osl],
                                             start=True, stop=False)
                            nc.tensor.matmul(pso[:, 0:512], ctxseg[1][u][:, csl_],
                                             wo_sb[1][:, osl],
                                             start=False, stop=True)
                            osb = obp.tile([128, 512], F32, tag="ob", bufs=4,
                                           name="osb")
                            route_copy(osb, pso[:, 0:512], 512, 1,
                                       nc.vector.tensor_copy, nc.scalar.copy)
                            nc.sync.dma_start(out_d[ssl, osl], osb)

        for _rep in range(reps):
            if _rep:
                tc.strict_bb_all_engine_barrier()
            _one_pass(_rep)

    nc.compile()
    return nc


def _get_nc(reps=1):
    key = ("nc", reps, _BUILD_STAGES)
    if key not in _cached:
        _cached[key] = _build_nc(reps)
    return _cached[key]


def _fp22(a):
    """Truncate fp32 to fp22 (e8m13) as the PE's float32r datapath does."""
    a = np.ascontiguousarray(a, dtype=np.float32)
    a.view(np.uint32)[...] &= np.uint32(0xFFFFFC00)
    return a


def _host_inputs(query, key, value, Wq, Wk, Wv, Wo):
    """Build the 8 per-core input dicts (host-side transposes/slices)."""
    f32 = np.float32
    xt = {}
    for b in range(B):
        xt[("q", b)] = _fp22(query[b].T)
        xt[("k", b)] = _fp22(key[b].T)
        xt[("v", b)] = _fp22(value[b].T)
    import ml_dtypes

    cmask = np.where(
        np.arange(128)[None, :] <= np.arange(128)[:, None], 0.0, _MASKVAL
    ).astype(f32)
    ident = np.eye(128).astype(ml_dtypes.bfloat16)
    in_maps = []
    for c in range(8):
        b, g = c // 4, c % 4
        jsl = slice(JC * g, JC * (g + 1))
        in_maps.append(
            {
                "xtq": xt[("q", b)],
                "xtk": xt[("k", b)],
                "xtv": xt[("v", b)],
                "wqt": _fp22(Wq[jsl, :].T),
                "wkt": _fp22(Wk[jsl, :].T),
                "wvt": _fp22(Wv[jsl, :].T),
                "wot": _fp22(Wo[:, jsl].T),
                "cmask": cmask,
                "ident": ident,
            }
        )
    return in_maps


def _numpy_fallback(query, key, value, mask, Wq, Wk, Wv, Wo):
    """Exact (chunked) numpy path for non-causal masks."""
    out = np.empty((B, S, D), dtype=np.float32)
    q = (query @ Wq.T).reshape(B, S, H, DK).transpose(0, 2, 1, 3)
    k = (key @ Wk.T).reshape(B, S, H, DK).transpose(0, 2, 1, 3)
    v = (value @ Wv.T).reshape(B, S, H, DK).transpose(0, 2, 1, 3)
    for b in range(B):
        ctx = np.empty((H, S, DK), dtype=np.float32)
        mb = mask[b] == 0
        for h in range(H):
            s = (q[b, h] @ k[b, h].T) * _SCALE
            s[mb] = np.finfo(np.float32).min
            s -= s.max(axis=1, keepdims=True)
            np.exp(s, out=s)
            s /= s.sum(axis=1, keepdims=True)
            ctx[h] = s @ v[b, h]
        out[b] = ctx.transpose(1, 0, 2).reshape(S, D) @ Wo.T
    return out


def kernel(query, key, value, mask, Wq, Wk, Wv, Wo):
    query = np.asarray(query, dtype=np.float32)
    key = np.asarray(key, dtype=np.float32)
    value = np.asarray(value, dtype=np.float32)
    mask = np.asarray(mask)
    Wq, Wk, Wv, Wo = (np.asarray(w, dtype=np.float32) for w in (Wq, Wk, Wv, Wo))

    tril = np.tril(np.ones((S, S), dtype=mask.dtype))
    if not all(np.array_equal(mask[b], tril) for b in range(B)):
        return _numpy_fallback(query, key, value, mask, Wq, Wk, Wv, Wo)

    from concourse.bass_utils import run_bass_kernel_spmd

    nc = _get_nc()
    in_maps = _host_inputs(query, key, value, Wq, Wk, Wv, Wo)
    res = run_bass_kernel_spmd(nc, in_maps, core_ids=list(range(8)))
    outs = [r["out"] for r in res.results]
    full = np.empty((B, S, D), dtype=np.float32)
    for b in range(B):
        full[b] = outs[4 * b] + outs[4 * b + 1] + outs[4 * b + 2] + outs[4 * b + 3]
    return full


# revision 16
# speedup vs baseline: 4.5533x; 4.5533x over previous
"""Multi-head causal attention (B=2, S=2048, D=1024, H=16) on 8 trn2 NeuronCores.

Sharding: data-parallel over batch (2) x tensor-parallel over heads (4 groups of
4 heads).  Core c = 4*b + g handles batch b, heads [4g, 4g+4).  Each core
computes a partial output  ctx_g @ Wo_g.T  [2048, 1024]; the host sums the 4
partials per batch.

Within-core dataflow:
  qT,kT = W @ X.T        float32r matmuls (fp32 operands, fp22 datapath, full
                         PE rate at moving-dim >= 256); [dk, s] pair-packed
  v     = X @ Wv.T       natural [s, dv], stored bf16
  S'    = -8 * (qT.T @ kT + mask)   per 128-row query tile, causally-trimmed
                         1024-wide PSUM chunks; two heads row-packed via
                         tile_position.  The -8 scale + mask add + running
                         row min (= -8*max) come from one chained
                         tensor_tensor_reduce per chunk (DVE).
  P     = exp(-S' - 8m)  one ScalarE activation per chunk (scale=-1,
                         bias = chained min), accum_out gives row sums Z;
                         P written bf16.  Global row max => no rescale pass.
  P    *= 1/Z            per chunk; DVE (4x bf16) for late tiles, GPSIMD for
                         early ones (Pool engine otherwise idle)
  P_T   = PE transpose   bf16 128x128 blocks; both heads packed into one
                         [128,1024] PSUM tile per key tile, copied to SBUF
                         in one wide copy (engine chosen by a build-time
                         DVE/ACT cost balancer)
  ctxT  = v.T @ P_T      bf16, two heads col-packed, accumulated over k tiles
  out  += ctxT.T @ WoT   float32r, per s-tile, stored straight from PSUM
"""

import numpy as np

B, S, D, H = 2, 2048, 1024, 16
DK = D // H          # 64
JC = 256             # per-core projection width (4 heads * 64)
NQT = S // 128       # 16 query tiles
NU4 = S // 512       # 4 query supertiles
_SCALE = float(DK) ** 0.5  # 8.0  (reference multiplies scores by sqrt(dk))
_MASKVAL = -1.0e30
_BIG = 3.0e38
CW = 512              # score chunk width (PSUM bank-local)
PSS_BUFS = 6
USE_TTR = False

_cached = {}
_BUILD_STAGES = "full"  # debug: dma | proj | full


def _build_nc(reps=1):
    stages = _BUILD_STAGES
    from contextlib import ExitStack

    import concourse.mybir as mybir
    import concourse.tile as tile
    from concourse import bacc

    F32 = mybir.dt.float32
    F32R = mybir.dt.float32r
    BF16 = mybir.dt.bfloat16
    EXP = mybir.ActivationFunctionType.Exp
    AX = mybir.AxisListType.X
    MIN = mybir.AluOpType.min
    MAX = mybir.AluOpType.max
    ADD = mybir.AluOpType.add

    nc = bacc.Bacc("TRN2", target_bir_lowering=False)

    xtq_d = nc.dram_tensor("xtq", [D, S], F32R, kind="ExternalInput")
    xtk_d = nc.dram_tensor("xtk", [D, S], F32R, kind="ExternalInput")
    xtv_d = nc.dram_tensor("xtv", [D, S], F32R, kind="ExternalInput")
    wqt_d = nc.dram_tensor("wqt", [D, JC], F32R, kind="ExternalInput")
    wkt_d = nc.dram_tensor("wkt", [D, JC], F32R, kind="ExternalInput")
    wvt_d = nc.dram_tensor("wvt", [D, JC], F32R, kind="ExternalInput")
    wot_d = nc.dram_tensor("wot", [JC, D], F32R, kind="ExternalInput")
    cmask_d = nc.dram_tensor("cmask", [128, 128], F32, kind="ExternalInput")
    ident_d = nc.dram_tensor("ident", [128, 128], BF16, kind="ExternalInput")
    out_d = nc.dram_tensor("out", [S, D], F32, kind="ExternalOutput")

    # build-time engine busy estimate (ns) for greedy op routing.
    # DVE: (120+FD/accel)/0.96 from PSUM; ACT: (172+FD)/1.2 (no 16-bit
    # packing observed on ScalarE); Pool: 95 + 1.67*FD/1.2 SBUF-only.
    bal = {"v": 0.0, "a": 0.0, "p": 0.0}

    def route_copy(dst, src, fd, accel_v, vec, act):
        cv = (120 + fd / accel_v) / 0.96
        ca = (172 + fd) / 1.2
        if bal["v"] + cv <= bal["a"] + ca:
            bal["v"] += cv
            vec(dst, src)
        else:
            bal["a"] += ca
            act(dst, src)

    with tile.TileContext(nc) as tc, ExitStack() as top:
        res = top.enter_context(tc.tile_pool(name="res", bufs=1))
        stats = top.enter_context(tc.tile_pool(name="stats", bufs=1))

        # ---- resident tiles -------------------------------------------------
        # weight layouts: w?_sb[p, dt, j] = W?T[128*dt + p, j]
        wq_sb = res.tile([128, 8, JC], F32R, tag="wq")
        wk_sb = res.tile([128, 8, JC], F32R, tag="wk")
        wv_sb = res.tile([128, 8, JC], F32R, tag="wv")
        nc.sync.dma_start(wq_sb, wqt_d[:, :].rearrange("(t p) j -> p t j", p=128))
        nc.sync.dma_start(wk_sb, wkt_d[:, :].rearrange("(t p) j -> p t j", p=128))
        nc.sync.dma_start(wv_sb, wvt_d[:, :].rearrange("(t p) j -> p t j", p=128))
        wo_sb = []
        for p2 in range(2):
            t = res.tile([128, D], F32R, tag=f"wo{p2}", name=f"wo{p2}")
            nc.sync.dma_start(t, wot_d[128 * p2 : 128 * (p2 + 1), :])
            wo_sb.append(t)
        cmask = res.tile([128, 128], F32, tag="cmask")
        ident = res.tile([128, 128], BF16, tag="ident")
        nc.sync.dma_start(cmask, cmask_d[:, :])
        nc.sync.dma_start(ident, ident_d[:, :])
        zeros = res.tile([128, 1024], F32, tag="zeros")
        nc.scalar.memzero(zeros[:, :])

        # projected tensors (resident through attention), segmented 512-wide so
        # Tile's per-tile dependency tracking lets attention start on early
        # segments while later projection chunks are still in flight
        qseg = [[res.tile([128, 512], F32R, tag=f"qts{i}{c}", name=f"qts{i}{c}")
                 for c in range(4)] for i in range(2)]
        kseg = [[res.tile([128, 512], F32R, tag=f"kts{i}{c}", name=f"kts{i}{c}")
                 for c in range(4)] for i in range(2)]
        vu = [res.tile([128, JC], BF16, tag=f"vu{i}", name=f"vu{i}") for i in range(NQT)]
        ctxseg = [[res.tile([128, 512], F32R, tag=f"ctx{i}{c}", name=f"ctx{i}{c}")
                   for c in range(4)] for i in range(2)]

        def _one_pass(_rep):
            # ---- stage B: projections --------------------------------------
            with ExitStack() as stage_b:
                xpool = stage_b.enter_context(tc.tile_pool(name=f"xt{_rep}", bufs=1))
                pjp = stage_b.enter_context(
                    tc.tile_pool(name=f"pj{_rep}", bufs=1, space="PSUM")
                )
                CH = 512
                work = [(xtq_d, "q", 0), (xtk_d, "k", 0), (xtq_d, "q", 1),
                        (xtk_d, "k", 1), (xtq_d, "q", 2), (xtk_d, "k", 2),
                        (xtq_d, "q", 3), (xtk_d, "k", 3)]
                work += [(xtv_d, "v", c) for c in range(4)]
                for n_, (xd, kind, ch) in enumerate(work):
                    sl = slice(ch * CH, (ch + 1) * CH)
                    xc = xpool.tile([128, 8, CH], F32R, tag="xc", bufs=3, name="xc")
                    dmae = nc.sync if n_ % 2 == 0 else nc.scalar
                    dmae.dma_start(
                        xc, xd[:, sl].rearrange("(t p) s -> p t s", p=128)
                    )
                    if stages == "dma":
                        dd = stats.tile([128, 1], F32, tag="dd", bufs=8, name="dd")
                        nc.vector.reduce_max(
                            out=dd, in_=xc[:, 0, :].bitcast(F32), axis=AX
                        )
                        nc.sync.dma_start(out_d[0:128, n_ : n_ + 1], dd)
                        continue
                    if kind in ("q", "k"):
                        wsb = wq_sb if kind == "q" else wk_sb
                        dst = qseg if kind == "q" else kseg
                        for jt in range(2):
                            ps = pjp.tile([128, CH], F32, tag="pj", bufs=3,
                                          name="psqk")
                            for dt in range(8):
                                nc.tensor.matmul(
                                    ps,
                                    wsb[:, dt, 128 * jt : 128 * (jt + 1)],
                                    xc[:, dt, :],
                                    start=(dt == 0),
                                    stop=(dt == 7),
                                )
                            route_copy(dst[jt][ch], ps, 512, 1,
                                       nc.vector.tensor_copy, nc.scalar.copy)
                    else:
                        for st in range(4):
                            ps = pjp.tile([128, JC], F32, tag="pjv", bufs=2,
                                          name="psv")
                            for dt in range(8):
                                nc.tensor.matmul(
                                    ps,
                                    xc[:, dt, st * 128 : (st + 1) * 128],
                                    wv_sb[:, dt, :],
                                    start=(dt == 0),
                                    stop=(dt == 7),
                                )
                            route_copy(vu[4 * ch + st], ps, 256, 2,
                                       nc.vector.tensor_copy, nc.scalar.copy)

            if stages == "dma":
                return
            if stages == "proj":
                nc.sync.dma_start(out_d[0:128, :], qseg[0][0][:, 0:1024].bitcast(F32))
                nc.sync.dma_start(out_d[128:256, :], kseg[1][0][:, 0:1024].bitcast(F32))
                return

            # ---- stage C/D: attention + output projection ------------------
            with ExitStack() as stage_c:
                ppool = stage_c.enter_context(tc.tile_pool(name=f"pp{_rep}", bufs=1))
                ptp = stage_c.enter_context(tc.tile_pool(name=f"ptp{_rep}", bufs=1))
                obp = stage_c.enter_context(tc.tile_pool(name=f"obp{_rep}", bufs=1))
                pss_p = stage_c.enter_context(
                    tc.tile_pool(name=f"pss{_rep}", bufs=1, space="PSUM"))
                pst_p = stage_c.enter_context(
                    tc.tile_pool(name=f"pst{_rep}", bufs=1, space="PSUM"))
                psc_p = stage_c.enter_context(
                    tc.tile_pool(name=f"psc{_rep}", bufs=1, space="PSUM"))

                for u in range(NU4):
                    for p in range(2):
                        ptiles = {}
                        for sq in range(4):
                            qi = 4 * u + sq
                            W = 128 * (qi + 1)          # causal row width
                            nch = (W + CW - 1) // CW    # CW-wide chunks
                            for h in range(2):
                                hsl = slice(64 * h, 64 * (h + 1))
                                pt = ppool.tile([128, S], BF16, tag="P", bufs=16,
                                                name=f"P{p}{sq}{h}")
                                ptiles[(h, sq)] = pt
                                stt = stats.tile([128, 12], F32, tag="st", bufs=16,
                                                 name="stt")
                                chunks = []
                                k = 0  # running-min chain slot
                                for c in range(nch):
                                    off = CW * c
                                    wc = min(W - off, CW)
                                    ps = pss_p.tile([128, CW], F32, tag="pss",
                                                    bufs=PSS_BUFS, name="pss")
                                    chunks.append((ps, off, wc))
                                    for j in range((wc + 511) // 512):
                                        soff = off + 512 * j
                                        wj = min(wc - 512 * j, 512)
                                        nc.tensor.matmul(
                                            ps[:, 512 * j : 512 * j + wj],
                                            qseg[p][u][hsl, 128 * sq : 128 * sq + 128],
                                            kseg[p][soff // 512][hsl, 0:wj],
                                            start=True,
                                            stop=True,
                                            tile_position=(64 * h, 0),
                                        )
                                    # fold mask + *(-8) + running row-min
                                    # (= -8*rowmax) into chained TTRs
                                    last = c == nch - 1
                                    ndw = wc - 128 if last else wc
                                    if USE_TTR:
                                        if ndw > 0:
                                            nc.vector.tensor_tensor_reduce(
                                                out=ps[:, 0:ndw], in0=ps[:, 0:ndw],
                                                in1=zeros[:, 0:ndw], scale=-_SCALE,
                                                scalar=(_BIG if k == 0
                                                        else stt[:, k - 1 : k]),
                                                op0=ADD, op1=MIN,
                                                accum_out=stt[:, k : k + 1])
                                            bal["v"] += (120 + ndw) / 0.96
                                            k += 1
                                        if last:
                                            nc.vector.tensor_tensor_reduce(
                                                out=ps[:, ndw:wc], in0=ps[:, ndw:wc],
                                                in1=cmask, scale=-_SCALE,
                                                scalar=(_BIG if k == 0
                                                        else stt[:, k - 1 : k]),
                                                op0=ADD, op1=MIN,
                                                accum_out=stt[:, k : k + 1])
                                            bal["v"] += (120 + 128) / 0.96
                                            k += 1
                                    else:
                                        if last:
                                            nc.vector.tensor_add(
                                                ps[:, ndw:wc], ps[:, ndw:wc],
                                                cmask)
                                            bal["v"] += (120 + 128) / 0.96
                                        nc.vector.reduce_max(
                                            out=stt[:, k : k + 1],
                                            in_=ps[:, 0:wc], axis=AX)
                                        bal["v"] += (120 + wc) / 0.96
                                        if k > 0:
                                            nc.vector.tensor_tensor(
                                                out=stt[:, k : k + 1],
                                                in0=stt[:, k - 1 : k],
                                                in1=stt[:, k : k + 1],
                                                op=MAX)
                                        k += 1
                                if USE_TTR:
                                    mfin = stt[:, k - 1 : k]
                                    esc = -1.0
                                else:
                                    nc.vector.tensor_scalar_mul(
                                        stt[:, k : k + 1], stt[:, k - 1 : k],
                                        -_SCALE)
                                    mfin = stt[:, k : k + 1]
                                    esc = _SCALE
                                for c, (ps, off, wc) in enumerate(chunks):
                                    nc.scalar.activation(
                                        out=pt[:, off : off + wc],
                                        in_=ps[:, 0:wc],
                                        func=EXP,
                                        bias=mfin,
                                        scale=esc,
                                        accum_out=stt[:, 5 + c : 6 + c],
                                    )
                                    bal["a"] += (172 + wc + 187) / 1.2
                                if nch > 1:
                                    nc.vector.reduce_sum(
                                        out=stt[:, 9:10], in_=stt[:, 5 : 5 + nch],
                                        axis=AX)
                                    zsum = stt[:, 9:10]
                                else:
                                    zsum = stt[:, 5:6]
                                nc.vector.reciprocal(stt[:, 10:11], zsum)
                                rz = stt[:, 10:11]
                                bal["v"] += 130.0
                                for c, (ps, off, wc) in enumerate(chunks):
                                    cv = (58 + wc / 4) / 0.96
                                    cp = 95 + 1.67 * wc / 1.2
                                    if False and bal["p"] + cp <= bal["v"] + cv:
                                        bal["p"] += cp
                                        nc.gpsimd.tensor_scalar_mul(
                                            pt[:, off : off + wc],
                                            pt[:, off : off + wc], rz)
                                    else:
                                        bal["v"] += cv
                                        nc.vector.tensor_scalar_mul(
                                            pt[:, off : off + wc],
                                            pt[:, off : off + wc], rz)

                        # transposes + PV for this (pair, supertile)
                        psc = psc_p.tile([128, 512], F32, tag="psc", bufs=1,
                                         name=f"psc{p}{u}")
                        for t in range(4 * u + 4):
                            vstart = max(0, t - 4 * u)
                            pstile = pst_p.tile([128, 1024], BF16, tag="pst",
                                                bufs=1, name="pst")
                            for h in range(2):
                                for sq in range(vstart, 4):
                                    nc.tensor.transpose(
                                        pstile[:, 512 * h + 128 * sq
                                               : 512 * h + 128 * (sq + 1)],
                                        ptiles[(h, sq)][:, 128 * t : 128 * (t + 1)],
                                        ident,
                                    )
                            ptsb = ptp.tile([128, 1024], BF16, tag="pt", bufs=3,
                                            name="ptsb")
                            if vstart == 0:
                                route_copy(ptsb, pstile, 1024, 2,
                                           nc.vector.tensor_copy, nc.scalar.copy)
                            else:
                                for h in range(2):
                                    csl = slice(512 * h + 128 * vstart,
                                                512 * h + 512)
                                    route_copy(ptsb[:, csl], pstile[:, csl],
                                               512 - 128 * vstart, 2,
                                               nc.vector.tensor_copy,
                                               nc.scalar.copy)
                            for h in range(2):
                                csl = slice(128 * vstart, 512)
                                nc.tensor.matmul(
                                    psc[64 * h : 64 * (h + 1), csl],
                                    vu[t][:, 64 * (2 * p + h) : 64 * (2 * p + h + 1)],
                                    ptsb[:, 512 * h + 128 * vstart : 512 * h + 512],
                                    start=(t == 0),
                                    stop=(t == 4 * u + 3),
                                    tile_position=(0, 64 * h),
                                    skip_group_check=True,
                                )
                        route_copy(ctxseg[p][u], psc, 512, 1,
                                   nc.vector.tensor_copy, nc.scalar.copy)

                    # output projection for the four finished s-tiles
                    for st_ in range(4 * u, 4 * u + 4):
                        ssl = slice(128 * st_, 128 * (st_ + 1))
                        csl_ = slice(128 * (st_ % 4), 128 * (st_ % 4) + 128)
                        for oc in range(2):
                            osl = slice(512 * oc, 512 * (oc + 1))
                            pso = pss_p.tile([128, CW], F32, tag="pss",
                                             bufs=PSS_BUFS, name="pso")
                            nc.tensor.matmul(pso[:, 0:512], ctxseg[0][u][:, csl_],
                                             wo_sb[0][:, osl],
                                             start=True, stop=False)
                            nc.tensor.matmul(pso[:, 0:512], ctxseg[1][u][:, csl_],
                                             wo_sb[1][:, osl],
                                             start=False, stop=True)
                            osb = obp.tile([128, 512], F32, tag="ob", bufs=4,
                                           name="osb")
                            route_copy(osb, pso[:, 0:512], 512, 1,
                                       nc.vector.tensor_copy, nc.scalar.copy)
                            nc.sync.dma_start(out_d[ssl, osl], osb)

        for _rep in range(reps):
            if _rep:
                tc.strict_bb_all_engine_barrier()
            _one_pass(_rep)

    nc.compile()
    return nc


def _get_nc(reps=1):
    key = ("nc", reps, _BUILD_STAGES)
    if key not in _cached:
        _cached[key] = _build_nc(reps)
    return _cached[key]


def _fp22(a):
    """Truncate fp32 to fp22 (e8m13) as the PE's float32r datapath does."""
    a = np.ascontiguousarray(a, dtype=np.float32)
    a.view(np.uint32)[...] &= np.uint32(0xFFFFFC00)
    return a


def _host_inputs(query, key, value, Wq, Wk, Wv, Wo):
    """Build the 8 per-core input dicts (host-side transposes/slices)."""
    f32 = np.float32
    xt = {}
    for b in range(B):
        xt[("q", b)] = _fp22(query[b].T)
        xt[("k", b)] = _fp22(key[b].T)
        xt[("v", b)] = _fp22(value[b].T)
    import ml_dtypes

    cmask = np.where(
        np.arange(128)[None, :] <= np.arange(128)[:, None], 0.0, _MASKVAL
    ).astype(f32)
    ident = np.eye(128).astype(ml_dtypes.bfloat16)
    in_maps = []
    for c in range(8):
        b, g = c // 4, c % 4
        jsl = slice(JC * g, JC * (g + 1))
        in_maps.append(
            {
                "xtq": xt[("q", b)],
                "xtk": xt[("k", b)],
                "xtv": xt[("v", b)],
                "wqt": _fp22(Wq[jsl, :].T),
                "wkt": _fp22(Wk[jsl, :].T),
                "wvt": _fp22(Wv[jsl, :].T),
                "wot": _fp22(Wo[:, jsl].T),
                "cmask": cmask,
                "ident": ident,
            }
        )
    return in_maps


def _numpy_fallback(query, key, value, mask, Wq, Wk, Wv, Wo):
    """Exact (chunked) numpy path for non-causal masks."""
    out = np.empty((B, S, D), dtype=np.float32)
    q = (query @ Wq.T).reshape(B, S, H, DK).transpose(0, 2, 1, 3)
    k = (key @ Wk.T).reshape(B, S, H, DK).transpose(0, 2, 1, 3)
    v = (value @ Wv.T).reshape(B, S, H, DK).transpose(0, 2, 1, 3)
    for b in range(B):
        ctx = np.empty((H, S, DK), dtype=np.float32)
        mb = mask[b] == 0
        for h in range(H):
            s = (q[b, h] @ k[b, h].T) * _SCALE
            s[mb] = np.finfo(np.float32).min
            s -= s.max(axis=1, keepdims=True)
            np.exp(s, out=s)
            s /= s.sum(axis=1, keepdims=True)
            ctx[h] = s @ v[b, h]
        out[b] = ctx.transpose(1, 0, 2).reshape(S, D) @ Wo.T
    return out


def kernel(query, key, value, mask, Wq, Wk, Wv, Wo):
    query = np.asarray(query, dtype=np.float32)
    key = np.asarray(key, dtype=np.float32)
    value = np.asarray(value, dtype=np.float32)
    mask = np.asarray(mask)
    Wq, Wk, Wv, Wo = (np.asarray(w, dtype=np.float32) for w in (Wq, Wk, Wv, Wo))

    tril = np.tril(np.ones((S, S), dtype=mask.dtype))
    if not all(np.array_equal(mask[b], tril) for b in range(B)):
        return _numpy_fallback(query, key, value, mask, Wq, Wk, Wv, Wo)

    from concourse.bass_utils import run_bass_kernel_spmd

    nc = _get_nc()
    in_maps = _host_inputs(query, key, value, Wq, Wk, Wv, Wo)
    res = run_bass_kernel_spmd(nc, in_maps, core_ids=list(range(8)))
    outs = [r["out"] for r in res.results]
    full = np.empty((B, S, D), dtype=np.float32)
    for b in range(B):
        full[b] = outs[4 * b] + outs[4 * b + 1] + outs[4 * b + 2] + outs[4 * b + 3]
    return full
